# revision 53
# baseline (speedup 1.0000x reference)
"""MoE (top-1 routing) expert-parallel kernel for 8 TRN2 NeuronCores.

Strategy
--------
Expert parallelism with host-side dispatch/combine:
  - Host sorts tokens by expert id (stable argsort, same permutation the
    reference uses), slices the sorted stream into one contiguous block per
    expert, pads each block to a static capacity CAP, and transposes to
    [D, CAP] so the device kernel needs no on-chip transposes.
  - Both probability scalings of the reference fold into a single per-token
    input scale when biases are zero and probabilities are non-negative:
        out[perm[j]] = MLP_e(x[perm[j]] * p[perm[j]]) * p[j]
                     = MLP_e(x[perm[j]] * p[perm[j]] * p[j])
    (relu(s*a) == s*relu(a) for s >= 0, and both GEMMs are linear).
  - Core e runs a dense 2-layer MLP for expert e in bf16 (fp32 accumulate):
        H^T = relu(W1^T X^T), Y^T = W2^T H^T
    keeping everything in the [feature, token] layout so layer-1 output
    feeds layer 2 directly as the moving operand.
  - Host scatters per-expert outputs back to token order.

The device kernel is compiled once per (capacity, bias) variant and cached.
"""

import math
import os
from contextlib import ExitStack

import ml_dtypes
import numpy as np

import concourse.bass as bass  # noqa: F401  (bass types used via bacc/tile)
import concourse.tile as tile
from concourse import bacc, mybir
from concourse.bass_utils import run_bass_kernel_spmd

NCORES = 8
P = 128  # SBUF partitions

_BF16 = ml_dtypes.bfloat16
_CACHE: dict = {}
# Two device-kernel builders exist: the Tile-scheduled one (default) and a
# hand-scheduled Block-mode one (MOE_RAW=1). They measure within noise of
# each other (~74us); Tile is the default for robustness.
# The raw Block-mode builder predates the packed input layout — keep it
# disabled (it would misread the packed arrays).
_USE_RAW = False


def _tok_tiles(cap: int):
    """Near-equal token tiles, each <=512 (one PSUM bank) and 16-aligned.

    Equal-ish tiles keep every matmul's free dim large enough that the
    (hidden) LDWEIGHTS never becomes the issue-rate bound, unlike a
    512/512/remainder split whose tiny tail tile is LDW-bound.
    """
    ntiles = max(1, math.ceil(cap / 512))
    base = min(512, math.ceil(cap / ntiles / 16) * 16)
    sizes = []
    left = cap
    for _ in range(ntiles):
        tn = min(base, left)
        sizes.append(tn)
        left -= tn
    assert left == 0 and all(tn > 0 for tn in sizes)
    tiles = []
    t0 = 0
    for tn in sizes:
        tiles.append((t0, tn))
        t0 += tn
    return tiles


def _build(cap: int, d: int, f: int, use_bias: bool):
    """Dense per-core expert MLP: yT = W2^T relu(W1^T xT (+b1)) (+b2)."""
    kd, kf = d // P, f // P
    bf = mybir.dt.bfloat16
    f32 = mybir.dt.float32

    nc = bacc.Bacc("TRN2", target_bir_lowering=False, debug=False,
                   num_devices=NCORES)
    # Inputs arrive host-packed in SBUF consumption order (see kernel()):
    # every input DMA is then a plain 2D slice with 2-8KB contiguous DRAM
    # runs instead of 512B-1KB strided runs — ~2x better queue efficiency,
    # which directly moves the first-matmul gate earlier.
    xT = nc.dram_tensor("xT", [P, kd * cap], bf, kind="ExternalInput").ap()
    w1 = nc.dram_tensor("w1", [P, kd * f], bf, kind="ExternalInput").ap()
    w2 = nc.dram_tensor("w2", [P, kf * d], bf, kind="ExternalInput").ap()
    if use_bias:
        b1 = nc.dram_tensor("b1", [P, kf], f32, kind="ExternalInput").ap()
        b2 = nc.dram_tensor("b2", [P, kd], f32, kind="ExternalInput").ap()
    yT = nc.dram_tensor("yT", [d, cap], f32, kind="ExternalOutput").ap()

    yTv = yT.rearrange("(k p) t -> k p t", p=P)

    relu = mybir.ActivationFunctionType.Relu
    ident = mybir.ActivationFunctionType.Identity

    with tile.TileContext(nc) as tc, ExitStack() as ctx:
        wp = ctx.enter_context(tc.tile_pool(name="weights", bufs=1))
        hp = ctx.enter_context(tc.tile_pool(name="h", bufs=1))
        pp = ctx.enter_context(tc.tile_pool(name="psum", bufs=7, space="PSUM"))
        wpp = ctx.enter_context(tc.tile_pool(name="wpsum", bufs=1,
                                             space="PSUM"))
        op = ctx.enter_context(tc.tile_pool(name="out", bufs=4))

        tok_tiles = _tok_tiles(cap)

        # Free-dim base offset of each token tile in the packed x layout.
        x_off = []
        base = 0
        for (_, tn) in tok_tiles:
            x_off.append(base)
            base += kd * tn
        assert base == kd * cap

        x_sb = wp.tile([P, kd * cap], bf, tag="x", name="x")
        w1_sb = wp.tile([P, kd * f], bf, tag="w1", name="w1")
        w2_sb = wp.tile([P, kf * d], bf, tag="w2", name="w2")

        _, tn_first = tok_tiles[0]
        # Consumption order: x tok-tile 0, W1 in m-group chunks, remaining
        # x tok-tiles, then W2 (needed only once layer 2 starts).
        nc.sync.dma_start(x_sb[:, :kd * tn_first], xT[:, :kd * tn_first])
        c0 = 0
        for m_chunk in (2, 2, 4, kf - 8):
            cols = m_chunk * kd * P
            nc.sync.dma_start(w1_sb[:, c0:c0 + cols], w1[:, c0:c0 + cols])
            c0 += cols
        assert c0 == kd * f
        if tn_first < cap:
            nc.sync.dma_start(x_sb[:, kd * tn_first:], xT[:, kd * tn_first:])
        half = kf * d // 2
        nc.sync.dma_start(w2_sb[:, :half], w2[:, :half])
        nc.sync.dma_start(w2_sb[:, half:], w2[:, half:])
        if use_bias:
            b1_sb = wp.tile([P, kf], f32, tag="b1")
            nc.sync.dma_start(b1_sb[:], b1[:])
            b2_sb = wp.tile([P, kd], f32, tag="b2")
            nc.sync.dma_start(b2_sb[:], b2[:])

        h_sb = [hp.tile([P, cap], bf, tag=f"h_{m}", name=f"h_{m}")
                for m in range(kf)]

        # HAM warm-up: the PE clock-gate only opens to 2.4GHz after ~3.4us of
        # sustained matmul activity. The PE sits idle waiting for the first
        # input DMAs anyway, so burn that window on dummy matmuls over a
        # zeroed scratch tile; real matmuls then run warm from the start.
        warm = wp.tile([P, 512], bf, tag="warm", name="warm")
        nc.gpsimd.memset(warm[:], 0.0)
        wps = wpp.tile([P, 512], f32, tag="wps", name="wps")
        for _ in range(6):
            nc.tensor.matmul(wps[:], warm[:, :P], warm[:], start=True,
                             stop=True)

        # All of layer 1 first (its W1/x deps arrive early), then layer 2.
        for ti, (t0, tn) in enumerate(tok_tiles):
            for m in range(kf):
                ps = pp.tile([P, 512], f32, tag="ps")
                for k in range(kd):
                    w1c = (m * kd + k) * P
                    xc = x_off[ti] + k * tn
                    nc.tensor.matmul(
                        ps[:, :tn],
                        w1_sb[:, w1c:w1c + P],
                        x_sb[:, xc:xc + tn],
                        start=(k == 0),
                        stop=(k == kd - 1),
                    )
                bias = b1_sb[:, m:m + 1] if use_bias else 0.0
                nc.scalar.activation(h_sb[m][:, t0:t0 + tn], ps[:, :tn],
                                     relu, bias=bias)
        l2_groups = [(t0, tn, dm) for (t0, tn) in tok_tiles
                     for dm in range(kd)]
        for gi, (t0, tn, dm) in enumerate(l2_groups):
            # Split the very last group in half: the first half's
            # copy+DMA-out then overlaps the second half's matmuls, so the
            # output chain exposed after the final matmul is halved.
            if gi == len(l2_groups) - 1 and tn >= 128:
                hn = (tn // 2 + 15) // 16 * 16
                parts = [(t0, hn), (t0 + hn, tn - hn)]
            else:
                parts = [(t0, tn)]
            for (s0, sn) in parts:
                ps = pp.tile([P, 512], f32, tag="ps")
                for k in range(kf):
                    w2c = (dm * kf + k) * P
                    nc.tensor.matmul(
                        ps[:, :sn],
                        w2_sb[:, w2c:w2c + P],
                        h_sb[k][:, s0:s0 + sn],
                        start=(k == 0),
                        stop=(k == kf - 1),
                    )
                ot = op.tile([P, 512], f32, tag="ot")
                if use_bias:
                    nc.scalar.activation(ot[:, :sn], ps[:, :sn], ident,
                                         bias=b2_sb[:, dm:dm + 1])
                else:
                    nc.vector.tensor_copy(ot[:, :sn], ps[:, :sn])
                nc.sync.dma_start(yTv[dm][:, s0:s0 + sn], ot[:, :sn])

    nc.compile()
    return nc


def _build_raw(cap: int, d: int, f: int):
    """Hand-scheduled Block-mode variant of _build (no biases).

    Skips TileContext's all-engine barriers, per-tile semaphore machinery
    and the kernel-tail drain/clear/barrier butterfly (~15us of fixed
    overhead): engines synchronize through five manually-placed semaphores.

    Group order g: 0..2*kf-1 are layer-1 (t = g//kf, m = g%kf) producing
    relu'd h tiles; then 2*kd groups of layer-2 (t, dm) whose PSUM is
    copied to SBUF and DMA'd out. PSUM rotates through a 7-bank ring;
    producer waits for the slot's previous consumer via the shared
    `sem_done` count (consumed groups form a prefix of the group order,
    so count >= g-6 implies slot g-7 is free).
    """
    kd, kf = d // P, f // P
    bf = mybir.dt.bfloat16
    f32 = mybir.dt.float32
    tok_tiles = _tok_tiles(cap)
    nt = len(tok_tiles)
    n_l1 = nt * kf
    n_l2 = nt * kd
    RING = 7

    nc = bacc.Bacc("TRN2", target_bir_lowering=False, debug=False,
                   num_devices=NCORES)
    xT = nc.dram_tensor("xT", [d, cap], bf, kind="ExternalInput").ap()
    w1 = nc.dram_tensor("w1", [d, f], bf, kind="ExternalInput").ap()
    w2 = nc.dram_tensor("w2", [f, d], bf, kind="ExternalInput").ap()
    yT = nc.dram_tensor("yT", [d, cap], f32, kind="ExternalOutput").ap()
    yTv = yT.rearrange("(k p) t -> k p t", p=P)
    x_src = xT.rearrange("(k p) t -> p k t", p=P)
    w1_src = w1.rearrange("(k p) f -> p k f", p=P)
    w2_src = w2.rearrange("(k p) d -> p k d", p=P)

    relu = mybir.ActivationFunctionType.Relu

    with ExitStack() as ctx:
        x_sb = ctx.enter_context(nc.sbuf_tensor("x_sb", [P, kd * cap], bf))
        w1_sb = ctx.enter_context(nc.sbuf_tensor("w1_sb", [P, kd * f], bf))
        w2_sb = ctx.enter_context(nc.sbuf_tensor("w2_sb", [P, kf * d], bf))
        h_sb = ctx.enter_context(nc.sbuf_tensor("h_sb", [P, kf * cap], bf))
        o_sb = ctx.enter_context(nc.sbuf_tensor("o_sb", [P, n_l2 * 512], f32))
        warm = ctx.enter_context(nc.sbuf_tensor("warm", [P, 512], bf))
        ps = [ctx.enter_context(nc.psum_tensor(f"ps{i}", [P, 512], f32))
              for i in range(RING)]
        wps = ctx.enter_context(nc.psum_tensor("wps", [P, 512], f32))
        sem_in = ctx.enter_context(nc.semaphore("sem_in"))
        sem_mm = ctx.enter_context(nc.semaphore("sem_mm"))
        sem_done = ctx.enter_context(nc.semaphore("sem_done"))
        sem_out = ctx.enter_context(nc.semaphore("sem_out"))

        x_dst = x_sb.ap().rearrange("p (k t) -> p k t", k=kd)
        w1_dst = w1_sb.ap().rearrange("p (k f) -> p k f", k=kd)
        w2_dst = w2_sb.ap().rearrange("p (k d) -> p k d", k=kf)

        # Input DMA order (each +16 on sem_in): x tok0; W1 in 4 f-chunks;
        # remaining x; W2 in 2 halves.  dma_need[g] = sem_in level gating
        # group g's first matmul.
        t0f, tnf = tok_tiles[0]
        w1_chunks = (256, 256, 512, 1024)

        def l1_need(t, m):
            """sem_in level required before L1 group (tile t, m-block m)."""
            acc = 0
            chunk_hi = len(w1_chunks)
            for ci, w in enumerate(w1_chunks):
                acc += w
                if (m + 1) * P <= acc:
                    chunk_hi = ci + 1
                    break
            w1_lvl = 1 + chunk_hi  # x-tok0 + W1 chunks up to chunk_hi
            if t == 0:
                return 16 * w1_lvl
            # tile t's x arrives as DMA #(1 + nchunks + t)
            return 16 * max(w1_lvl, 1 + len(w1_chunks) + t)

        n_in_dma = 1 + len(w1_chunks) + (nt - 1) + 2

        with nc.Block() as block:

            @block.sync
            def _(sync):
                sync.dma_start(x_dst[:, :, :tnf],
                               x_src[:, :, :tnf]).then_inc(sem_in, 16)
                c0 = 0
                for w in w1_chunks:
                    sync.dma_start(w1_dst[:, :, c0:c0 + w],
                                   w1_src[:, :, c0:c0 + w]).then_inc(sem_in, 16)
                    c0 += w
                for (t0, tn) in tok_tiles[1:]:
                    sync.dma_start(x_dst[:, :, t0:t0 + tn],
                                   x_src[:, :, t0:t0 + tn]).then_inc(sem_in, 16)
                half = kf // 2
                sync.dma_start(w2_dst[:, :half, :],
                               w2_src[:, :half, :]).then_inc(sem_in, 16)
                sync.dma_start(w2_dst[:, half:, :],
                               w2_src[:, half:, :]).then_inc(sem_in, 16)
                # outputs: group j done when sem_done reaches n_l1 + j + 1
                for j in range(n_l2):
                    t = j // kd
                    dm = j % kd
                    t0, tn = tok_tiles[t]
                    sync.wait_ge(sem_done, n_l1 + j + 1)
                    sync.dma_start(yTv[dm][:, t0:t0 + tn],
                                   o_sb[:, j * 512:j * 512 + tn]
                                   ).then_inc(sem_out, 16)
                sync.wait_ge(sem_out, 16 * n_l2)

            @block.tensor
            def _(tensor):
                # HAM warm-up on garbage data while input DMAs stream.
                for _ in range(8):
                    tensor.matmul(wps[:], warm[:, :P], warm[:],
                                  start=True, stop=True)
                g = 0
                for ti, (t0, tn) in enumerate(tok_tiles):
                    for m in range(kf):
                        pst = ps[g % RING]
                        if g >= RING:
                            tensor.wait_ge(sem_done, g - RING + 1)
                        tensor.wait_ge(sem_in, l1_need(ti, m))
                        for k in range(kd):
                            mm = tensor.matmul(
                                pst[:, :tn],
                                w1_sb[:, k * f + m * P:k * f + (m + 1) * P],
                                x_sb[:, k * cap + t0:k * cap + t0 + tn],
                                start=(k == 0),
                                stop=(k == kd - 1),
                            )
                            if k == kd - 1:
                                mm.then_inc(sem_mm, 1)
                        g += 1
                for ti, (t0, tn) in enumerate(tok_tiles):
                    for dm in range(kd):
                        pst = ps[g % RING]
                        # ring slot free AND this tile's relus all written
                        tensor.wait_ge(sem_done,
                                       max(g - RING + 1, (ti + 1) * kf))
                        tensor.wait_ge(sem_in, 16 * n_in_dma)
                        for k in range(kf):
                            mm = tensor.matmul(
                                pst[:, :tn],
                                w2_sb[:, k * d + dm * P:k * d + (dm + 1) * P],
                                h_sb[:, k * cap + t0:k * cap + t0 + tn],
                                start=(k == 0),
                                stop=(k == kf - 1),
                            )
                            if k == kf - 1:
                                mm.then_inc(sem_mm, 1)
                        g += 1

            @block.scalar
            def _(scalar):
                g = 0
                for (t0, tn) in tok_tiles:
                    for m in range(kf):
                        scalar.wait_ge(sem_mm, g + 1)
                        scalar.activation(
                            h_sb[:, m * cap + t0:m * cap + t0 + tn],
                            ps[g % RING][:, :tn], relu,
                        ).then_inc(sem_done, 1)
                        g += 1

            @block.vector
            def _(vector):
                for j in range(n_l2):
                    g = n_l1 + j
                    t = j // kd
                    t0, tn = tok_tiles[t]
                    vector.wait_ge(sem_mm, g + 1)
                    vector.tensor_copy(
                        o_sb[:, j * 512:j * 512 + tn],
                        ps[g % RING][:, :tn],
                    ).then_inc(sem_done, 1)

            @block.gpsimd
            def _(gpsimd):
                # Reset semaphores so a re-execution of the loaded NEFF
                # starts from a clean state.
                gpsimd.wait_ge(sem_out, 16 * n_l2)
                for s in (sem_in, sem_mm, sem_done, sem_out):
                    gpsimd.sem_clear(s)

    nc.compile()
    return nc


def _get_nc(cap: int, d: int, f: int, use_bias: bool):
    key = (cap, d, f, use_bias)
    if key not in _CACHE:
        if use_bias or not _USE_RAW:
            _CACHE[key] = _build(cap, d, f, use_bias)
        else:
            _CACHE[key] = _build_raw(cap, d, f)
    return _CACHE[key]


def kernel(input_batch, probabilities, W1, b1, W2, b2, indices):
    x = np.ascontiguousarray(np.asarray(input_batch, dtype=np.float32))
    p = np.asarray(probabilities, dtype=np.float32)
    W1 = np.asarray(W1, dtype=np.float32)
    b1 = np.asarray(b1, dtype=np.float32)
    W2 = np.asarray(W2, dtype=np.float32)
    b2 = np.asarray(b2, dtype=np.float32)
    idx = np.asarray(indices)

    n, d = x.shape
    e_num, _, f = W1.shape
    assert e_num <= NCORES, "one expert per core"
    assign = idx[:, 0].astype(np.int64)

    # Stable sort by expert — the same grouping order the reference's
    # argsort produces, so position j in the sorted stream carries the
    # reference's post-scale p[j].
    order = np.argsort(assign, kind="stable")
    a_sorted = assign[order]
    eids = np.arange(e_num)
    starts = np.searchsorted(a_sorted, eids, side="left")
    ends = np.searchsorted(a_sorted, eids, side="right")
    counts = ends - starts
    maxc = int(counts.max()) if e_num else 0

    # Fixed capacity N/e keeps the device shape input-independent and the
    # tensor engine stream minimal; the few tokens above capacity in
    # over-full experts are computed on the host (tiny GEMMs). If the
    # routing is pathologically skewed, fall back to a full-capacity device
    # kernel instead of a big host GEMM.
    cap = max(512, int(math.ceil(n / max(e_num, 1) / 512)) * 512)
    overflow_total = int(np.maximum(counts - cap, 0).sum())
    if overflow_total > 512:
        cap = int(math.ceil(max(maxc, 16) / 16)) * 16

    use_bias = bool(np.any(b1)) or bool(np.any(b2))
    fold = (not use_bias) and bool(np.all(p >= 0))

    pre = p[order]
    post = p[:n]  # reference applies p in *sequential* order to sorted rows
    scale = pre * post if fold else pre
    xs = x[order] * scale[:, None]

    dev_counts = np.minimum(counts, cap)
    kd, kf = d // P, f // P
    tiles = _tok_tiles(cap)

    # Pack every input in the device kernel's SBUF consumption order so each
    # DMA is a contiguous 2D slice (2-8KB DRAM runs): x is (tile, k, token),
    # W1 is (m, k, col) m-major, W2 is (dm, k, col) dm-major.
    xT = np.zeros((NCORES, d, cap), dtype=_BF16)
    for e in range(e_num):
        blk = xs[starts[e]:starts[e] + dev_counts[e]]
        if blk.shape[0]:
            xT[e, :, :dev_counts[e]] = blk.T.astype(_BF16)
    xr = xT.reshape(NCORES, kd, P, cap)
    xp = np.concatenate(
        [xr[:, :, :, t0:t0 + tn].transpose(0, 2, 1, 3).reshape(NCORES, P, kd * tn)
         for (t0, tn) in tiles], axis=2)
    xp = np.ascontiguousarray(xp)
    w1b = np.ascontiguousarray(
        W1.astype(_BF16).reshape(e_num, kd, P, kf, P)
        .transpose(0, 2, 3, 1, 4).reshape(e_num, P, kf * kd * P))
    w2b = np.ascontiguousarray(
        W2.astype(_BF16).reshape(e_num, kf, P, kd, P)
        .transpose(0, 2, 3, 1, 4).reshape(e_num, P, kd * kf * P))

    nc = _get_nc(cap, d, f, use_bias)
    in_maps = []
    for c in range(NCORES):
        e = min(c, e_num - 1)  # replicate last expert on spare cores
        m = {"xT": xp[c] if c < e_num else xp[0],
             "w1": w1b[e], "w2": w2b[e]}
        if use_bias:
            m["b1"] = np.ascontiguousarray(b1[e].reshape(f // P, P).T)
            m["b2"] = np.ascontiguousarray(b2[e].reshape(d // P, P).T)
        in_maps.append(m)

    global _last_in_maps
    _last_in_maps = in_maps
    res = run_bass_kernel_spmd(nc, in_maps, core_ids=list(range(NCORES)))

    y_sorted = np.zeros((n, d), dtype=np.float32)
    for e in range(e_num):
        if dev_counts[e]:
            yT = res.results[e]["yT"]
            y_sorted[starts[e]:starts[e] + dev_counts[e]] = \
                yT[:, :dev_counts[e]].T
        if counts[e] > dev_counts[e]:  # overflow tokens: host GEMM
            sl = slice(starts[e] + dev_counts[e], ends[e])
            h = np.maximum(xs[sl] @ W1[e] + b1[e], 0.0)
            y_sorted[sl] = h @ W2[e] + b2[e]
    if not fold:
        y_sorted *= post[:, None]

    out = np.empty((n, d), dtype=np.float32)
    out[order] = y_sorted
    total_loss = np.asarray(0.0, dtype=np.float32)
    return out, total_loss


# revision 55
# speedup vs baseline: 1.0109x; 1.0109x over previous
"""MoE (top-1 routing) expert-parallel kernel for 8 TRN2 NeuronCores.

Strategy
--------
Expert parallelism with host-side dispatch/combine:
  - Host sorts tokens by expert id (stable argsort, same permutation the
    reference uses), slices the sorted stream into one contiguous block per
    expert, pads each block to a static capacity CAP, and transposes to
    [D, CAP] so the device kernel needs no on-chip transposes.
  - Both probability scalings of the reference fold into a single per-token
    input scale when biases are zero and probabilities are non-negative:
        out[perm[j]] = MLP_e(x[perm[j]] * p[perm[j]]) * p[j]
                     = MLP_e(x[perm[j]] * p[perm[j]] * p[j])
    (relu(s*a) == s*relu(a) for s >= 0, and both GEMMs are linear).
  - Core e runs a dense 2-layer MLP for expert e in bf16 (fp32 accumulate):
        H^T = relu(W1^T X^T), Y^T = W2^T H^T
    keeping everything in the [feature, token] layout so layer-1 output
    feeds layer 2 directly as the moving operand.
  - Host scatters per-expert outputs back to token order.

The device kernel is compiled once per (capacity, bias) variant and cached.
"""

import math
import os
from contextlib import ExitStack

import ml_dtypes
import numpy as np

import concourse.bass as bass  # noqa: F401  (bass types used via bacc/tile)
import concourse.tile as tile
from concourse import bacc, mybir
from concourse.bass_utils import run_bass_kernel_spmd

NCORES = 8
P = 128  # SBUF partitions

_BF16 = ml_dtypes.bfloat16
_CACHE: dict = {}
# Two device-kernel builders exist: the Tile-scheduled one (default) and a
# hand-scheduled Block-mode one (MOE_RAW=1). They measure within noise of
# each other (~74us); Tile is the default for robustness.
# The raw Block-mode builder predates the packed input layout — keep it
# disabled (it would misread the packed arrays).
_USE_RAW = False


def _tok_tiles(cap: int):
    """Near-equal token tiles, each <=512 (one PSUM bank) and 16-aligned.

    Equal-ish tiles keep every matmul's free dim large enough that the
    (hidden) LDWEIGHTS never becomes the issue-rate bound, unlike a
    512/512/remainder split whose tiny tail tile is LDW-bound.
    """
    ntiles = max(1, math.ceil(cap / 512))
    base = min(512, math.ceil(cap / ntiles / 16) * 16)
    sizes = []
    left = cap
    for _ in range(ntiles):
        tn = min(base, left)
        sizes.append(tn)
        left -= tn
    assert left == 0 and all(tn > 0 for tn in sizes)
    tiles = []
    t0 = 0
    for tn in sizes:
        tiles.append((t0, tn))
        t0 += tn
    return tiles


def _build(cap: int, d: int, f: int, use_bias: bool):
    """Dense per-core expert MLP: yT = W2^T relu(W1^T xT (+b1)) (+b2)."""
    kd, kf = d // P, f // P
    bf = mybir.dt.bfloat16
    f32 = mybir.dt.float32

    nc = bacc.Bacc("TRN2", target_bir_lowering=False, debug=False,
                   num_devices=NCORES)
    # Inputs arrive host-packed in SBUF consumption order (see kernel()):
    # every input DMA is then a plain 2D slice with 2-8KB contiguous DRAM
    # runs instead of 512B-1KB strided runs — ~2x better queue efficiency,
    # which directly moves the first-matmul gate earlier.
    xT = nc.dram_tensor("xT", [P, kd * cap], bf, kind="ExternalInput").ap()
    w1 = nc.dram_tensor("w1", [P, kd * f], bf, kind="ExternalInput").ap()
    w2 = nc.dram_tensor("w2", [P, kf * d], bf, kind="ExternalInput").ap()
    if use_bias:
        b1 = nc.dram_tensor("b1", [P, kf], f32, kind="ExternalInput").ap()
        b2 = nc.dram_tensor("b2", [P, kd], f32, kind="ExternalInput").ap()
    yT = nc.dram_tensor("yT", [d, cap], f32, kind="ExternalOutput").ap()

    yTv = yT.rearrange("(k p) t -> k p t", p=P)

    relu = mybir.ActivationFunctionType.Relu
    ident = mybir.ActivationFunctionType.Identity

    with tile.TileContext(nc) as tc, ExitStack() as ctx:
        wp = ctx.enter_context(tc.tile_pool(name="weights", bufs=1))
        hp = ctx.enter_context(tc.tile_pool(name="h", bufs=1))
        pp = ctx.enter_context(tc.tile_pool(name="psum", bufs=7, space="PSUM"))
        wpp = ctx.enter_context(tc.tile_pool(name="wpsum", bufs=1,
                                             space="PSUM"))
        op = ctx.enter_context(tc.tile_pool(name="out", bufs=4))

        tok_tiles = _tok_tiles(cap)

        # Free-dim base offset of each token tile in the packed x layout.
        x_off = []
        base = 0
        for (_, tn) in tok_tiles:
            x_off.append(base)
            base += kd * tn
        assert base == kd * cap

        x_sb = wp.tile([P, kd * cap], bf, tag="x", name="x")
        w1_sb = wp.tile([P, kd * f], bf, tag="w1", name="w1")
        w2_sb = wp.tile([P, kf * d], bf, tag="w2", name="w2")

        _, tn_first = tok_tiles[0]
        # Consumption order: x tok-tile 0, W1 in m-group chunks, remaining
        # x tok-tiles, then W2 (needed only once layer 2 starts).
        nc.sync.dma_start(x_sb[:, :kd * tn_first], xT[:, :kd * tn_first])
        c0 = 0
        for m_chunk in (2, 2, 4, kf - 8):
            cols = m_chunk * kd * P
            nc.sync.dma_start(w1_sb[:, c0:c0 + cols], w1[:, c0:c0 + cols])
            c0 += cols
        assert c0 == kd * f
        if tn_first < cap:
            nc.sync.dma_start(x_sb[:, kd * tn_first:], xT[:, kd * tn_first:])
        # Packed W2 is one contiguous region; one DMA (full W2 arrives
        # ~20us before layer 2 first needs it, so coarse granularity is
        # free and saves an issue slot).
        nc.sync.dma_start(w2_sb[:], w2[:])
        if use_bias:
            b1_sb = wp.tile([P, kf], f32, tag="b1")
            nc.sync.dma_start(b1_sb[:], b1[:])
            b2_sb = wp.tile([P, kd], f32, tag="b2")
            nc.sync.dma_start(b2_sb[:], b2[:])

        h_sb = [hp.tile([P, cap], bf, tag=f"h_{m}", name=f"h_{m}")
                for m in range(kf)]

        # HAM warm-up: the PE clock-gate only opens to 2.4GHz after ~3.4us of
        # sustained matmul activity. The PE sits idle waiting for the first
        # input DMAs anyway, so burn that window on dummy matmuls over a
        # zeroed scratch tile; real matmuls then run warm from the start.
        warm = wp.tile([P, 512], bf, tag="warm", name="warm")
        nc.gpsimd.memset(warm[:], 0.0)
        wps = wpp.tile([P, 512], f32, tag="wps", name="wps")
        # 8 dummies (~5us) deliberately over-bridge: input delivery varies
        # 11-14us core-to-core, and a too-short bridge risks a >3.4us PE gap
        # and a HAM re-throttle (expensive); overshooting costs <=1us.
        for _ in range(8):
            nc.tensor.matmul(wps[:], warm[:, :P], warm[:], start=True,
                             stop=True)

        # All of layer 1 first (its W1/x deps arrive early), then layer 2.
        for ti, (t0, tn) in enumerate(tok_tiles):
            for m in range(kf):
                ps = pp.tile([P, 512], f32, tag="ps")
                for k in range(kd):
                    w1c = (m * kd + k) * P
                    xc = x_off[ti] + k * tn
                    nc.tensor.matmul(
                        ps[:, :tn],
                        w1_sb[:, w1c:w1c + P],
                        x_sb[:, xc:xc + tn],
                        start=(k == 0),
                        stop=(k == kd - 1),
                    )
                bias = b1_sb[:, m:m + 1] if use_bias else 0.0
                nc.scalar.activation(h_sb[m][:, t0:t0 + tn], ps[:, :tn],
                                     relu, bias=bias)
        l2_groups = [(t0, tn, dm) for (t0, tn) in tok_tiles
                     for dm in range(kd)]
        for gi, (t0, tn, dm) in enumerate(l2_groups):
            # Split the very last group in half: the first half's
            # copy+DMA-out then overlaps the second half's matmuls, so the
            # output chain exposed after the final matmul is halved.
            if gi == len(l2_groups) - 1 and tn >= 128:
                hn = (tn // 2 + 15) // 16 * 16
                parts = [(t0, hn), (t0 + hn, tn - hn)]
            else:
                parts = [(t0, tn)]
            for (s0, sn) in parts:
                ps = pp.tile([P, 512], f32, tag="ps")
                for k in range(kf):
                    w2c = (dm * kf + k) * P
                    nc.tensor.matmul(
                        ps[:, :sn],
                        w2_sb[:, w2c:w2c + P],
                        h_sb[k][:, s0:s0 + sn],
                        start=(k == 0),
                        stop=(k == kf - 1),
                    )
                ot = op.tile([P, 512], f32, tag="ot")
                if use_bias:
                    nc.scalar.activation(ot[:, :sn], ps[:, :sn], ident,
                                         bias=b2_sb[:, dm:dm + 1])
                else:
                    nc.vector.tensor_copy(ot[:, :sn], ps[:, :sn])
                nc.sync.dma_start(yTv[dm][:, s0:s0 + sn], ot[:, :sn])

    nc.compile()
    return nc


def _build_raw(cap: int, d: int, f: int):
    """Hand-scheduled Block-mode variant of _build (no biases).

    Skips TileContext's all-engine barriers, per-tile semaphore machinery
    and the kernel-tail drain/clear/barrier butterfly (~15us of fixed
    overhead): engines synchronize through five manually-placed semaphores.

    Group order g: 0..2*kf-1 are layer-1 (t = g//kf, m = g%kf) producing
    relu'd h tiles; then 2*kd groups of layer-2 (t, dm) whose PSUM is
    copied to SBUF and DMA'd out. PSUM rotates through a 7-bank ring;
    producer waits for the slot's previous consumer via the shared
    `sem_done` count (consumed groups form a prefix of the group order,
    so count >= g-6 implies slot g-7 is free).
    """
    kd, kf = d // P, f // P
    bf = mybir.dt.bfloat16
    f32 = mybir.dt.float32
    tok_tiles = _tok_tiles(cap)
    nt = len(tok_tiles)
    n_l1 = nt * kf
    n_l2 = nt * kd
    RING = 7

    nc = bacc.Bacc("TRN2", target_bir_lowering=False, debug=False,
                   num_devices=NCORES)
    xT = nc.dram_tensor("xT", [d, cap], bf, kind="ExternalInput").ap()
    w1 = nc.dram_tensor("w1", [d, f], bf, kind="ExternalInput").ap()
    w2 = nc.dram_tensor("w2", [f, d], bf, kind="ExternalInput").ap()
    yT = nc.dram_tensor("yT", [d, cap], f32, kind="ExternalOutput").ap()
    yTv = yT.rearrange("(k p) t -> k p t", p=P)
    x_src = xT.rearrange("(k p) t -> p k t", p=P)
    w1_src = w1.rearrange("(k p) f -> p k f", p=P)
    w2_src = w2.rearrange("(k p) d -> p k d", p=P)

    relu = mybir.ActivationFunctionType.Relu

    with ExitStack() as ctx:
        x_sb = ctx.enter_context(nc.sbuf_tensor("x_sb", [P, kd * cap], bf))
        w1_sb = ctx.enter_context(nc.sbuf_tensor("w1_sb", [P, kd * f], bf))
        w2_sb = ctx.enter_context(nc.sbuf_tensor("w2_sb", [P, kf * d], bf))
        h_sb = ctx.enter_context(nc.sbuf_tensor("h_sb", [P, kf * cap], bf))
        o_sb = ctx.enter_context(nc.sbuf_tensor("o_sb", [P, n_l2 * 512], f32))
        warm = ctx.enter_context(nc.sbuf_tensor("warm", [P, 512], bf))
        ps = [ctx.enter_context(nc.psum_tensor(f"ps{i}", [P, 512], f32))
              for i in range(RING)]
        wps = ctx.enter_context(nc.psum_tensor("wps", [P, 512], f32))
        sem_in = ctx.enter_context(nc.semaphore("sem_in"))
        sem_mm = ctx.enter_context(nc.semaphore("sem_mm"))
        sem_done = ctx.enter_context(nc.semaphore("sem_done"))
        sem_out = ctx.enter_context(nc.semaphore("sem_out"))

        x_dst = x_sb.ap().rearrange("p (k t) -> p k t", k=kd)
        w1_dst = w1_sb.ap().rearrange("p (k f) -> p k f", k=kd)
        w2_dst = w2_sb.ap().rearrange("p (k d) -> p k d", k=kf)

        # Input DMA order (each +16 on sem_in): x tok0; W1 in 4 f-chunks;
        # remaining x; W2 in 2 halves.  dma_need[g] = sem_in level gating
        # group g's first matmul.
        t0f, tnf = tok_tiles[0]
        w1_chunks = (256, 256, 512, 1024)

        def l1_need(t, m):
            """sem_in level required before L1 group (tile t, m-block m)."""
            acc = 0
            chunk_hi = len(w1_chunks)
            for ci, w in enumerate(w1_chunks):
                acc += w
                if (m + 1) * P <= acc:
                    chunk_hi = ci + 1
                    break
            w1_lvl = 1 + chunk_hi  # x-tok0 + W1 chunks up to chunk_hi
            if t == 0:
                return 16 * w1_lvl
            # tile t's x arrives as DMA #(1 + nchunks + t)
            return 16 * max(w1_lvl, 1 + len(w1_chunks) + t)

        n_in_dma = 1 + len(w1_chunks) + (nt - 1) + 2

        with nc.Block() as block:

            @block.sync
            def _(sync):
                sync.dma_start(x_dst[:, :, :tnf],
                               x_src[:, :, :tnf]).then_inc(sem_in, 16)
                c0 = 0
                for w in w1_chunks:
                    sync.dma_start(w1_dst[:, :, c0:c0 + w],
                                   w1_src[:, :, c0:c0 + w]).then_inc(sem_in, 16)
                    c0 += w
                for (t0, tn) in tok_tiles[1:]:
                    sync.dma_start(x_dst[:, :, t0:t0 + tn],
                                   x_src[:, :, t0:t0 + tn]).then_inc(sem_in, 16)
                half = kf // 2
                sync.dma_start(w2_dst[:, :half, :],
                               w2_src[:, :half, :]).then_inc(sem_in, 16)
                sync.dma_start(w2_dst[:, half:, :],
                               w2_src[:, half:, :]).then_inc(sem_in, 16)
                # outputs: group j done when sem_done reaches n_l1 + j + 1
                for j in range(n_l2):
                    t = j // kd
                    dm = j % kd
                    t0, tn = tok_tiles[t]
                    sync.wait_ge(sem_done, n_l1 + j + 1)
                    sync.dma_start(yTv[dm][:, t0:t0 + tn],
                                   o_sb[:, j * 512:j * 512 + tn]
                                   ).then_inc(sem_out, 16)
                sync.wait_ge(sem_out, 16 * n_l2)

            @block.tensor
            def _(tensor):
                # HAM warm-up on garbage data while input DMAs stream.
                for _ in range(8):
                    tensor.matmul(wps[:], warm[:, :P], warm[:],
                                  start=True, stop=True)
                g = 0
                for ti, (t0, tn) in enumerate(tok_tiles):
                    for m in range(kf):
                        pst = ps[g % RING]
                        if g >= RING:
                            tensor.wait_ge(sem_done, g - RING + 1)
                        tensor.wait_ge(sem_in, l1_need(ti, m))
                        for k in range(kd):
                            mm = tensor.matmul(
                                pst[:, :tn],
                                w1_sb[:, k * f + m * P:k * f + (m + 1) * P],
                                x_sb[:, k * cap + t0:k * cap + t0 + tn],
                                start=(k == 0),
                                stop=(k == kd - 1),
                            )
                            if k == kd - 1:
                                mm.then_inc(sem_mm, 1)
                        g += 1
                for ti, (t0, tn) in enumerate(tok_tiles):
                    for dm in range(kd):
                        pst = ps[g % RING]
                        # ring slot free AND this tile's relus all written
                        tensor.wait_ge(sem_done,
                                       max(g - RING + 1, (ti + 1) * kf))
                        tensor.wait_ge(sem_in, 16 * n_in_dma)
                        for k in range(kf):
                            mm = tensor.matmul(
                                pst[:, :tn],
                                w2_sb[:, k * d + dm * P:k * d + (dm + 1) * P],
                                h_sb[:, k * cap + t0:k * cap + t0 + tn],
                                start=(k == 0),
                                stop=(k == kf - 1),
                            )
                            if k == kf - 1:
                                mm.then_inc(sem_mm, 1)
                        g += 1

            @block.scalar
            def _(scalar):
                g = 0
                for (t0, tn) in tok_tiles:
                    for m in range(kf):
                        scalar.wait_ge(sem_mm, g + 1)
                        scalar.activation(
                            h_sb[:, m * cap + t0:m * cap + t0 + tn],
                            ps[g % RING][:, :tn], relu,
                        ).then_inc(sem_done, 1)
                        g += 1

            @block.vector
            def _(vector):
                for j in range(n_l2):
                    g = n_l1 + j
                    t = j // kd
                    t0, tn = tok_tiles[t]
                    vector.wait_ge(sem_mm, g + 1)
                    vector.tensor_copy(
                        o_sb[:, j * 512:j * 512 + tn],
                        ps[g % RING][:, :tn],
                    ).then_inc(sem_done, 1)

            @block.gpsimd
            def _(gpsimd):
                # Reset semaphores so a re-execution of the loaded NEFF
                # starts from a clean state.
                gpsimd.wait_ge(sem_out, 16 * n_l2)
                for s in (sem_in, sem_mm, sem_done, sem_out):
                    gpsimd.sem_clear(s)

    nc.compile()
    return nc


def _get_nc(cap: int, d: int, f: int, use_bias: bool):
    key = (cap, d, f, use_bias)
    if key not in _CACHE:
        if use_bias or not _USE_RAW:
            _CACHE[key] = _build(cap, d, f, use_bias)
        else:
            _CACHE[key] = _build_raw(cap, d, f)
    return _CACHE[key]


def kernel(input_batch, probabilities, W1, b1, W2, b2, indices):
    x = np.ascontiguousarray(np.asarray(input_batch, dtype=np.float32))
    p = np.asarray(probabilities, dtype=np.float32)
    W1 = np.asarray(W1, dtype=np.float32)
    b1 = np.asarray(b1, dtype=np.float32)
    W2 = np.asarray(W2, dtype=np.float32)
    b2 = np.asarray(b2, dtype=np.float32)
    idx = np.asarray(indices)

    n, d = x.shape
    e_num, _, f = W1.shape
    assert e_num <= NCORES, "one expert per core"
    assign = idx[:, 0].astype(np.int64)

    # Stable sort by expert — the same grouping order the reference's
    # argsort produces, so position j in the sorted stream carries the
    # reference's post-scale p[j].
    order = np.argsort(assign, kind="stable")
    a_sorted = assign[order]
    eids = np.arange(e_num)
    starts = np.searchsorted(a_sorted, eids, side="left")
    ends = np.searchsorted(a_sorted, eids, side="right")
    counts = ends - starts
    maxc = int(counts.max()) if e_num else 0

    # Fixed capacity N/e keeps the device shape input-independent and the
    # tensor engine stream minimal; the few tokens above capacity in
    # over-full experts are computed on the host (tiny GEMMs). If the
    # routing is pathologically skewed, fall back to a full-capacity device
    # kernel instead of a big host GEMM.
    cap = max(512, int(math.ceil(n / max(e_num, 1) / 512)) * 512)
    overflow_total = int(np.maximum(counts - cap, 0).sum())
    if overflow_total > 512:
        cap = int(math.ceil(max(maxc, 16) / 16)) * 16

    use_bias = bool(np.any(b1)) or bool(np.any(b2))
    fold = (not use_bias) and bool(np.all(p >= 0))

    pre = p[order]
    post = p[:n]  # reference applies p in *sequential* order to sorted rows
    scale = pre * post if fold else pre
    xs = x[order] * scale[:, None]

    dev_counts = np.minimum(counts, cap)
    kd, kf = d // P, f // P
    tiles = _tok_tiles(cap)

    # Pack every input in the device kernel's SBUF consumption order so each
    # DMA is a contiguous 2D slice (2-8KB DRAM runs): x is (tile, k, token),
    # W1 is (m, k, col) m-major, W2 is (dm, k, col) dm-major.
    xT = np.zeros((NCORES, d, cap), dtype=_BF16)
    for e in range(e_num):
        blk = xs[starts[e]:starts[e] + dev_counts[e]]
        if blk.shape[0]:
            xT[e, :, :dev_counts[e]] = blk.T.astype(_BF16)
    xr = xT.reshape(NCORES, kd, P, cap)
    xp = np.concatenate(
        [xr[:, :, :, t0:t0 + tn].transpose(0, 2, 1, 3).reshape(NCORES, P, kd * tn)
         for (t0, tn) in tiles], axis=2)
    xp = np.ascontiguousarray(xp)
    w1b = np.ascontiguousarray(
        W1.astype(_BF16).reshape(e_num, kd, P, kf, P)
        .transpose(0, 2, 3, 1, 4).reshape(e_num, P, kf * kd * P))
    w2b = np.ascontiguousarray(
        W2.astype(_BF16).reshape(e_num, kf, P, kd, P)
        .transpose(0, 2, 3, 1, 4).reshape(e_num, P, kd * kf * P))

    nc = _get_nc(cap, d, f, use_bias)
    in_maps = []
    for c in range(NCORES):
        e = min(c, e_num - 1)  # replicate last expert on spare cores
        m = {"xT": xp[c] if c < e_num else xp[0],
             "w1": w1b[e], "w2": w2b[e]}
        if use_bias:
            m["b1"] = np.ascontiguousarray(b1[e].reshape(f // P, P).T)
            m["b2"] = np.ascontiguousarray(b2[e].reshape(d // P, P).T)
        in_maps.append(m)

    global _last_in_maps
    _last_in_maps = in_maps
    res = run_bass_kernel_spmd(nc, in_maps, core_ids=list(range(NCORES)))

    y_sorted = np.zeros((n, d), dtype=np.float32)
    for e in range(e_num):
        if dev_counts[e]:
            yT = res.results[e]["yT"]
            y_sorted[starts[e]:starts[e] + dev_counts[e]] = \
                yT[:, :dev_counts[e]].T
        if counts[e] > dev_counts[e]:  # overflow tokens: host GEMM
            sl = slice(starts[e] + dev_counts[e], ends[e])
            h = np.maximum(xs[sl] @ W1[e] + b1[e], 0.0)
            y_sorted[sl] = h @ W2[e] + b2[e]
    if not fold:
        y_sorted *= post[:, None]

    out = np.empty((n, d), dtype=np.float32)
    out[order] = y_sorted
    total_loss = np.asarray(0.0, dtype=np.float32)
    return out, total_loss


# revision 57
# speedup vs baseline: 1.0397x; 1.0285x over previous
"""MoE (top-1 routing) expert-parallel kernel for 8 TRN2 NeuronCores.

Strategy
--------
Expert parallelism with host-side dispatch/combine:
  - Host sorts tokens by expert id (stable argsort, same permutation the
    reference uses), slices the sorted stream into one contiguous block per
    expert, pads each block to a static capacity CAP, and transposes to
    [D, CAP] so the device kernel needs no on-chip transposes.
  - Both probability scalings of the reference fold into a single per-token
    input scale when biases are zero and probabilities are non-negative:
        out[perm[j]] = MLP_e(x[perm[j]] * p[perm[j]]) * p[j]
                     = MLP_e(x[perm[j]] * p[perm[j]] * p[j])
    (relu(s*a) == s*relu(a) for s >= 0, and both GEMMs are linear).
  - Core e runs a dense 2-layer MLP for expert e in bf16 (fp32 accumulate):
        H^T = relu(W1^T X^T), Y^T = W2^T H^T
    keeping everything in the [feature, token] layout so layer-1 output
    feeds layer 2 directly as the moving operand.
  - Host scatters per-expert outputs back to token order.

The device kernel is compiled once per (capacity, bias) variant and cached.
"""

import math
import os
from contextlib import ExitStack

import ml_dtypes
import numpy as np

import concourse.bass as bass  # noqa: F401  (bass types used via bacc/tile)
import concourse.tile as tile
from concourse import bacc, mybir
from concourse.bass_utils import run_bass_kernel_spmd

NCORES = 8
P = 128  # SBUF partitions

_BF16 = ml_dtypes.bfloat16
_CACHE: dict = {}
# Two device-kernel builders exist: the Tile-scheduled one (default) and a
# hand-scheduled Block-mode one (MOE_RAW=1). They measure within noise of
# each other (~74us); Tile is the default for robustness.
# The raw Block-mode builder predates the packed input layout — keep it
# disabled (it would misread the packed arrays).
_USE_RAW = False


def _tok_tiles(cap: int):
    """Near-equal token tiles, each <=512 (one PSUM bank) and 16-aligned.

    Equal-ish tiles keep every matmul's free dim large enough that the
    (hidden) LDWEIGHTS never becomes the issue-rate bound, unlike a
    512/512/remainder split whose tiny tail tile is LDW-bound.
    """
    ntiles = max(1, math.ceil(cap / 512))
    base = min(512, math.ceil(cap / ntiles / 16) * 16)
    sizes = []
    left = cap
    for _ in range(ntiles):
        tn = min(base, left)
        sizes.append(tn)
        left -= tn
    assert left == 0 and all(tn > 0 for tn in sizes)
    tiles = []
    t0 = 0
    for tn in sizes:
        tiles.append((t0, tn))
        t0 += tn
    return tiles


def _build(cap: int, d: int, f: int, use_bias: bool):
    """Dense per-core expert MLP: yT = W2^T relu(W1^T xT (+b1)) (+b2)."""
    kd, kf = d // P, f // P
    bf = mybir.dt.bfloat16
    f32 = mybir.dt.float32

    nc = bacc.Bacc("TRN2", target_bir_lowering=False, debug=False,
                   num_devices=NCORES)
    # Inputs arrive host-packed in SBUF consumption order (see kernel()):
    # every input DMA is then a plain 2D slice with 2-8KB contiguous DRAM
    # runs instead of 512B-1KB strided runs — ~2x better queue efficiency,
    # which directly moves the first-matmul gate earlier.
    xT = nc.dram_tensor("xT", [P, kd * cap], bf, kind="ExternalInput").ap()
    w1 = nc.dram_tensor("w1", [P, kd * f], bf, kind="ExternalInput").ap()
    w2 = nc.dram_tensor("w2", [P, kf * d], bf, kind="ExternalInput").ap()
    if use_bias:
        b1 = nc.dram_tensor("b1", [P, kf], f32, kind="ExternalInput").ap()
        b2 = nc.dram_tensor("b2", [P, kd], f32, kind="ExternalInput").ap()
    # bf16 output halves the write-back traffic and the tail-exposed DMA;
    # the host upcasts on gather. Error cost is ~2^-9 per element on top of
    # the existing bf16 compute error — well inside the tolerance.
    yT = nc.dram_tensor("yT", [d, cap], bf, kind="ExternalOutput").ap()

    yTv = yT.rearrange("(k p) t -> k p t", p=P)

    relu = mybir.ActivationFunctionType.Relu
    ident = mybir.ActivationFunctionType.Identity

    with tile.TileContext(nc) as tc, ExitStack() as ctx:
        wp = ctx.enter_context(tc.tile_pool(name="weights", bufs=1))
        hp = ctx.enter_context(tc.tile_pool(name="h", bufs=1))
        pp = ctx.enter_context(tc.tile_pool(name="psum", bufs=7, space="PSUM"))
        wpp = ctx.enter_context(tc.tile_pool(name="wpsum", bufs=1,
                                             space="PSUM"))
        op = ctx.enter_context(tc.tile_pool(name="out", bufs=4))

        tok_tiles = _tok_tiles(cap)

        # Free-dim base offset of each token tile in the packed x layout.
        x_off = []
        base = 0
        for (_, tn) in tok_tiles:
            x_off.append(base)
            base += kd * tn
        assert base == kd * cap

        x_sb = wp.tile([P, kd * cap], bf, tag="x", name="x")
        w1_sb = wp.tile([P, kd * f], bf, tag="w1", name="w1")
        w2_sb = wp.tile([P, kf * d], bf, tag="w2", name="w2")

        _, tn_first = tok_tiles[0]
        # Consumption order: x tok-tile 0, W1 in m-group chunks, remaining
        # x tok-tiles, then W2 (needed only once layer 2 starts).
        nc.sync.dma_start(x_sb[:, :kd * tn_first], xT[:, :kd * tn_first])
        c0 = 0
        for m_chunk in (2, 2, 4, kf - 8):
            cols = m_chunk * kd * P
            nc.sync.dma_start(w1_sb[:, c0:c0 + cols], w1[:, c0:c0 + cols])
            c0 += cols
        assert c0 == kd * f
        if tn_first < cap:
            nc.sync.dma_start(x_sb[:, kd * tn_first:], xT[:, kd * tn_first:])
        # Packed W2 is one contiguous region; one DMA (full W2 arrives
        # ~20us before layer 2 first needs it, so coarse granularity is
        # free and saves an issue slot).
        nc.sync.dma_start(w2_sb[:], w2[:])
        if use_bias:
            b1_sb = wp.tile([P, kf], f32, tag="b1")
            nc.sync.dma_start(b1_sb[:], b1[:])
            b2_sb = wp.tile([P, kd], f32, tag="b2")
            nc.sync.dma_start(b2_sb[:], b2[:])

        h_sb = [hp.tile([P, cap], bf, tag=f"h_{m}", name=f"h_{m}")
                for m in range(kf)]

        # HAM warm-up: the PE clock-gate only opens to 2.4GHz after ~3.4us of
        # sustained matmul activity. The PE sits idle waiting for the first
        # input DMAs anyway, so burn that window on dummy matmuls over a
        # zeroed scratch tile; real matmuls then run warm from the start.
        warm = wp.tile([P, 512], bf, tag="warm", name="warm")
        nc.gpsimd.memset(warm[:], 0.0)
        wps = wpp.tile([P, 512], f32, tag="wps", name="wps")
        # 8 dummies (~5us) deliberately over-bridge: input delivery varies
        # 11-14us core-to-core, and a too-short bridge risks a >3.4us PE gap
        # and a HAM re-throttle (expensive); overshooting costs <=1us.
        for _ in range(8):
            nc.tensor.matmul(wps[:], warm[:, :P], warm[:], start=True,
                             stop=True)

        # All of layer 1 first (its W1/x deps arrive early), then layer 2.
        for ti, (t0, tn) in enumerate(tok_tiles):
            for m in range(kf):
                ps = pp.tile([P, 512], f32, tag="ps")
                for k in range(kd):
                    w1c = (m * kd + k) * P
                    xc = x_off[ti] + k * tn
                    nc.tensor.matmul(
                        ps[:, :tn],
                        w1_sb[:, w1c:w1c + P],
                        x_sb[:, xc:xc + tn],
                        start=(k == 0),
                        stop=(k == kd - 1),
                    )
                bias = b1_sb[:, m:m + 1] if use_bias else 0.0
                nc.scalar.activation(h_sb[m][:, t0:t0 + tn], ps[:, :tn],
                                     relu, bias=bias)
        l2_groups = [(t0, tn, dm) for (t0, tn) in tok_tiles
                     for dm in range(kd)]
        for gi, (t0, tn, dm) in enumerate(l2_groups):
            # Split the very last group in half: the first half's
            # copy+DMA-out then overlaps the second half's matmuls, so the
            # output chain exposed after the final matmul is halved.
            if gi == len(l2_groups) - 1 and tn >= 128:
                hn = (tn // 2 + 15) // 16 * 16
                parts = [(t0, hn), (t0 + hn, tn - hn)]
            else:
                parts = [(t0, tn)]
            for (s0, sn) in parts:
                ps = pp.tile([P, 512], f32, tag="ps")
                for k in range(kf):
                    w2c = (dm * kf + k) * P
                    nc.tensor.matmul(
                        ps[:, :sn],
                        w2_sb[:, w2c:w2c + P],
                        h_sb[k][:, s0:s0 + sn],
                        start=(k == 0),
                        stop=(k == kf - 1),
                    )
                ot = op.tile([P, 512], bf, tag="ot")
                if use_bias:
                    nc.scalar.activation(ot[:, :sn], ps[:, :sn], ident,
                                         bias=b2_sb[:, dm:dm + 1])
                else:
                    nc.vector.tensor_copy(ot[:, :sn], ps[:, :sn])
                nc.sync.dma_start(yTv[dm][:, s0:s0 + sn], ot[:, :sn])

    nc.compile()
    return nc


def _build_raw(cap: int, d: int, f: int):
    """Hand-scheduled Block-mode variant of _build (no biases).

    Skips TileContext's all-engine barriers, per-tile semaphore machinery
    and the kernel-tail drain/clear/barrier butterfly (~15us of fixed
    overhead): engines synchronize through five manually-placed semaphores.

    Group order g: 0..2*kf-1 are layer-1 (t = g//kf, m = g%kf) producing
    relu'd h tiles; then 2*kd groups of layer-2 (t, dm) whose PSUM is
    copied to SBUF and DMA'd out. PSUM rotates through a 7-bank ring;
    producer waits for the slot's previous consumer via the shared
    `sem_done` count (consumed groups form a prefix of the group order,
    so count >= g-6 implies slot g-7 is free).
    """
    kd, kf = d // P, f // P
    bf = mybir.dt.bfloat16
    f32 = mybir.dt.float32
    tok_tiles = _tok_tiles(cap)
    nt = len(tok_tiles)
    n_l1 = nt * kf
    n_l2 = nt * kd
    RING = 7

    nc = bacc.Bacc("TRN2", target_bir_lowering=False, debug=False,
                   num_devices=NCORES)
    xT = nc.dram_tensor("xT", [d, cap], bf, kind="ExternalInput").ap()
    w1 = nc.dram_tensor("w1", [d, f], bf, kind="ExternalInput").ap()
    w2 = nc.dram_tensor("w2", [f, d], bf, kind="ExternalInput").ap()
    yT = nc.dram_tensor("yT", [d, cap], f32, kind="ExternalOutput").ap()
    yTv = yT.rearrange("(k p) t -> k p t", p=P)
    x_src = xT.rearrange("(k p) t -> p k t", p=P)
    w1_src = w1.rearrange("(k p) f -> p k f", p=P)
    w2_src = w2.rearrange("(k p) d -> p k d", p=P)

    relu = mybir.ActivationFunctionType.Relu

    with ExitStack() as ctx:
        x_sb = ctx.enter_context(nc.sbuf_tensor("x_sb", [P, kd * cap], bf))
        w1_sb = ctx.enter_context(nc.sbuf_tensor("w1_sb", [P, kd * f], bf))
        w2_sb = ctx.enter_context(nc.sbuf_tensor("w2_sb", [P, kf * d], bf))
        h_sb = ctx.enter_context(nc.sbuf_tensor("h_sb", [P, kf * cap], bf))
        o_sb = ctx.enter_context(nc.sbuf_tensor("o_sb", [P, n_l2 * 512], f32))
        warm = ctx.enter_context(nc.sbuf_tensor("warm", [P, 512], bf))
        ps = [ctx.enter_context(nc.psum_tensor(f"ps{i}", [P, 512], f32))
              for i in range(RING)]
        wps = ctx.enter_context(nc.psum_tensor("wps", [P, 512], f32))
        sem_in = ctx.enter_context(nc.semaphore("sem_in"))
        sem_mm = ctx.enter_context(nc.semaphore("sem_mm"))
        sem_done = ctx.enter_context(nc.semaphore("sem_done"))
        sem_out = ctx.enter_context(nc.semaphore("sem_out"))

        x_dst = x_sb.ap().rearrange("p (k t) -> p k t", k=kd)
        w1_dst = w1_sb.ap().rearrange("p (k f) -> p k f", k=kd)
        w2_dst = w2_sb.ap().rearrange("p (k d) -> p k d", k=kf)

        # Input DMA order (each +16 on sem_in): x tok0; W1 in 4 f-chunks;
        # remaining x; W2 in 2 halves.  dma_need[g] = sem_in level gating
        # group g's first matmul.
        t0f, tnf = tok_tiles[0]
        w1_chunks = (256, 256, 512, 1024)

        def l1_need(t, m):
            """sem_in level required before L1 group (tile t, m-block m)."""
            acc = 0
            chunk_hi = len(w1_chunks)
            for ci, w in enumerate(w1_chunks):
                acc += w
                if (m + 1) * P <= acc:
                    chunk_hi = ci + 1
                    break
            w1_lvl = 1 + chunk_hi  # x-tok0 + W1 chunks up to chunk_hi
            if t == 0:
                return 16 * w1_lvl
            # tile t's x arrives as DMA #(1 + nchunks + t)
            return 16 * max(w1_lvl, 1 + len(w1_chunks) + t)

        n_in_dma = 1 + len(w1_chunks) + (nt - 1) + 2

        with nc.Block() as block:

            @block.sync
            def _(sync):
                sync.dma_start(x_dst[:, :, :tnf],
                               x_src[:, :, :tnf]).then_inc(sem_in, 16)
                c0 = 0
                for w in w1_chunks:
                    sync.dma_start(w1_dst[:, :, c0:c0 + w],
                                   w1_src[:, :, c0:c0 + w]).then_inc(sem_in, 16)
                    c0 += w
                for (t0, tn) in tok_tiles[1:]:
                    sync.dma_start(x_dst[:, :, t0:t0 + tn],
                                   x_src[:, :, t0:t0 + tn]).then_inc(sem_in, 16)
                half = kf // 2
                sync.dma_start(w2_dst[:, :half, :],
                               w2_src[:, :half, :]).then_inc(sem_in, 16)
                sync.dma_start(w2_dst[:, half:, :],
                               w2_src[:, half:, :]).then_inc(sem_in, 16)
                # outputs: group j done when sem_done reaches n_l1 + j + 1
                for j in range(n_l2):
                    t = j // kd
                    dm = j % kd
                    t0, tn = tok_tiles[t]
                    sync.wait_ge(sem_done, n_l1 + j + 1)
                    sync.dma_start(yTv[dm][:, t0:t0 + tn],
                                   o_sb[:, j * 512:j * 512 + tn]
                                   ).then_inc(sem_out, 16)
                sync.wait_ge(sem_out, 16 * n_l2)

            @block.tensor
            def _(tensor):
                # HAM warm-up on garbage data while input DMAs stream.
                for _ in range(8):
                    tensor.matmul(wps[:], warm[:, :P], warm[:],
                                  start=True, stop=True)
                g = 0
                for ti, (t0, tn) in enumerate(tok_tiles):
                    for m in range(kf):
                        pst = ps[g % RING]
                        if g >= RING:
                            tensor.wait_ge(sem_done, g - RING + 1)
                        tensor.wait_ge(sem_in, l1_need(ti, m))
                        for k in range(kd):
                            mm = tensor.matmul(
                                pst[:, :tn],
                                w1_sb[:, k * f + m * P:k * f + (m + 1) * P],
                                x_sb[:, k * cap + t0:k * cap + t0 + tn],
                                start=(k == 0),
                                stop=(k == kd - 1),
                            )
                            if k == kd - 1:
                                mm.then_inc(sem_mm, 1)
                        g += 1
                for ti, (t0, tn) in enumerate(tok_tiles):
                    for dm in range(kd):
                        pst = ps[g % RING]
                        # ring slot free AND this tile's relus all written
                        tensor.wait_ge(sem_done,
                                       max(g - RING + 1, (ti + 1) * kf))
                        tensor.wait_ge(sem_in, 16 * n_in_dma)
                        for k in range(kf):
                            mm = tensor.matmul(
                                pst[:, :tn],
                                w2_sb[:, k * d + dm * P:k * d + (dm + 1) * P],
                                h_sb[:, k * cap + t0:k * cap + t0 + tn],
                                start=(k == 0),
                                stop=(k == kf - 1),
                            )
                            if k == kf - 1:
                                mm.then_inc(sem_mm, 1)
                        g += 1

            @block.scalar
            def _(scalar):
                g = 0
                for (t0, tn) in tok_tiles:
                    for m in range(kf):
                        scalar.wait_ge(sem_mm, g + 1)
                        scalar.activation(
                            h_sb[:, m * cap + t0:m * cap + t0 + tn],
                            ps[g % RING][:, :tn], relu,
                        ).then_inc(sem_done, 1)
                        g += 1

            @block.vector
            def _(vector):
                for j in range(n_l2):
                    g = n_l1 + j
                    t = j // kd
                    t0, tn = tok_tiles[t]
                    vector.wait_ge(sem_mm, g + 1)
                    vector.tensor_copy(
                        o_sb[:, j * 512:j * 512 + tn],
                        ps[g % RING][:, :tn],
                    ).then_inc(sem_done, 1)

            @block.gpsimd
            def _(gpsimd):
                # Reset semaphores so a re-execution of the loaded NEFF
                # starts from a clean state.
                gpsimd.wait_ge(sem_out, 16 * n_l2)
                for s in (sem_in, sem_mm, sem_done, sem_out):
                    gpsimd.sem_clear(s)

    nc.compile()
    return nc


def _get_nc(cap: int, d: int, f: int, use_bias: bool):
    key = (cap, d, f, use_bias)
    if key not in _CACHE:
        if use_bias or not _USE_RAW:
            _CACHE[key] = _build(cap, d, f, use_bias)
        else:
            _CACHE[key] = _build_raw(cap, d, f)
    return _CACHE[key]


def kernel(input_batch, probabilities, W1, b1, W2, b2, indices):
    x = np.ascontiguousarray(np.asarray(input_batch, dtype=np.float32))
    p = np.asarray(probabilities, dtype=np.float32)
    W1 = np.asarray(W1, dtype=np.float32)
    b1 = np.asarray(b1, dtype=np.float32)
    W2 = np.asarray(W2, dtype=np.float32)
    b2 = np.asarray(b2, dtype=np.float32)
    idx = np.asarray(indices)

    n, d = x.shape
    e_num, _, f = W1.shape
    assert e_num <= NCORES, "one expert per core"
    assign = idx[:, 0].astype(np.int64)

    # Stable sort by expert — the same grouping order the reference's
    # argsort produces, so position j in the sorted stream carries the
    # reference's post-scale p[j].
    order = np.argsort(assign, kind="stable")
    a_sorted = assign[order]
    eids = np.arange(e_num)
    starts = np.searchsorted(a_sorted, eids, side="left")
    ends = np.searchsorted(a_sorted, eids, side="right")
    counts = ends - starts
    maxc = int(counts.max()) if e_num else 0

    # Fixed capacity N/e keeps the device shape input-independent and the
    # tensor engine stream minimal; the few tokens above capacity in
    # over-full experts are computed on the host (tiny GEMMs). If the
    # routing is pathologically skewed, fall back to a full-capacity device
    # kernel instead of a big host GEMM.
    cap = max(512, int(math.ceil(n / max(e_num, 1) / 512)) * 512)
    overflow_total = int(np.maximum(counts - cap, 0).sum())
    if overflow_total > 512:
        cap = int(math.ceil(max(maxc, 16) / 16)) * 16

    use_bias = bool(np.any(b1)) or bool(np.any(b2))
    fold = (not use_bias) and bool(np.all(p >= 0))

    pre = p[order]
    post = p[:n]  # reference applies p in *sequential* order to sorted rows
    scale = pre * post if fold else pre
    xs = x[order] * scale[:, None]

    dev_counts = np.minimum(counts, cap)
    kd, kf = d // P, f // P
    tiles = _tok_tiles(cap)

    # Pack every input in the device kernel's SBUF consumption order so each
    # DMA is a contiguous 2D slice (2-8KB DRAM runs): x is (tile, k, token),
    # W1 is (m, k, col) m-major, W2 is (dm, k, col) dm-major.
    xT = np.zeros((NCORES, d, cap), dtype=_BF16)
    for e in range(e_num):
        blk = xs[starts[e]:starts[e] + dev_counts[e]]
        if blk.shape[0]:
            xT[e, :, :dev_counts[e]] = blk.T.astype(_BF16)
    xr = xT.reshape(NCORES, kd, P, cap)
    xp = np.concatenate(
        [xr[:, :, :, t0:t0 + tn].transpose(0, 2, 1, 3).reshape(NCORES, P, kd * tn)
         for (t0, tn) in tiles], axis=2)
    xp = np.ascontiguousarray(xp)
    w1b = np.ascontiguousarray(
        W1.astype(_BF16).reshape(e_num, kd, P, kf, P)
        .transpose(0, 2, 3, 1, 4).reshape(e_num, P, kf * kd * P))
    w2b = np.ascontiguousarray(
        W2.astype(_BF16).reshape(e_num, kf, P, kd, P)
        .transpose(0, 2, 3, 1, 4).reshape(e_num, P, kd * kf * P))

    nc = _get_nc(cap, d, f, use_bias)
    in_maps = []
    for c in range(NCORES):
        e = min(c, e_num - 1)  # replicate last expert on spare cores
        m = {"xT": xp[c] if c < e_num else xp[0],
             "w1": w1b[e], "w2": w2b[e]}
        if use_bias:
            m["b1"] = np.ascontiguousarray(b1[e].reshape(f // P, P).T)
            m["b2"] = np.ascontiguousarray(b2[e].reshape(d // P, P).T)
        in_maps.append(m)

    global _last_in_maps
    _last_in_maps = in_maps
    res = run_bass_kernel_spmd(nc, in_maps, core_ids=list(range(NCORES)))

    y_sorted = np.zeros((n, d), dtype=np.float32)
    for e in range(e_num):
        if dev_counts[e]:
            yT = res.results[e]["yT"]
            y_sorted[starts[e]:starts[e] + dev_counts[e]] = \
                yT[:, :dev_counts[e]].T
        if counts[e] > dev_counts[e]:  # overflow tokens: host GEMM
            sl = slice(starts[e] + dev_counts[e], ends[e])
            h = np.maximum(xs[sl] @ W1[e] + b1[e], 0.0)
            y_sorted[sl] = h @ W2[e] + b2[e]
    if not fold:
        y_sorted *= post[:, None]

    out = np.empty((n, d), dtype=np.float32)
    out[order] = y_sorted
    total_loss = np.asarray(0.0, dtype=np.float32)
    return out, total_loss
